# revision 4
# baseline (speedup 1.0000x reference)
"""Conv4d (B=2, Ci=32, Co=64, 16^4 spatial, k=3^4, stride 1, pad 1) on 8
Trainium2 NeuronCores.

Sharding: 8 cores = batch(2) x T-quarters(4). Each core computes
out[64co, 4t, 16d, 16h, 16w] for its (b, t-quarter).

Per-core layout: SBUF x tile [128, 6t*6d*324] where partition group
r in {0..3} holds ci=32 channels of the padded input restricted to the
D-halo window [4r, 4r+6) (plus T halo), planes flattened as 18x18=324.
The 4 partition groups process the 4 output-D-quarters concurrently via
PE row-group tiling (tile_position=(32r, 0)).

Each output (t, d-pair) plane-pair accumulates 81 tap matmuls
(K=32ci, M=64co, N=512=2d*16h*16w) in fp32r (TF32) into one PSUM bank
per row group; epilogue adds bias (DVE/ACT tensor_scalar) and DMAs out.
"""
import sys

sys.path.insert(0, "/opt/trn_rl_repo")
import numpy as np

N_CORES = 8
TAPS = [(kt, kd, kh, kw) for kt in range(3) for kd in range(3)
        for kh in range(3) for kw in range(3)]

_NC = None


def _build():
    global _NC
    if _NC is not None:
        return _NC
    import concourse.bacc as bacc
    import concourse.tile as tile
    from concourse import mybir

    f32 = mybir.dt.float32
    f32r = mybir.dt.float32r

    nc = bacc.Bacc("TRN2", debug=False, target_bir_lowering=False,
                   num_devices=N_CORES)
    xq = nc.dram_tensor("xq", [128, 6 * 6 * 324], f32r, kind="ExternalInput")
    wq = nc.dram_tensor("wq", [32, 81 * 64], f32r, kind="ExternalInput")
    bq = nc.dram_tensor("biasq", [64, 1], f32, kind="ExternalInput")
    out = nc.dram_tensor("out", [64, 16384], f32, kind="ExternalOutput")

    with tile.TileContext(nc) as tc:
        with tc.tile_pool(name="xp", bufs=1) as xp, \
             tc.tile_pool(name="wp", bufs=1) as wp, \
             tc.tile_pool(name="op", bufs=6) as op_, \
             tc.tile_pool(name="pp", bufs=8, space="PSUM") as pp:
            # weights first: they gate every matmul.  Load one 0.66MB copy,
            # replicate to the other 3 partition groups on-chip.
            wtile = wp.tile([128, 5184], f32r)
            nc.gpsimd.dma_start(wtile[0:32, :], wq.ap()[:])
            btile = wp.tile([64, 1], f32)
            nc.gpsimd.dma_start(btile[:], bq.ap()[:])
            for r in range(1, 4):
                nc.gpsimd.dma_start(wtile[32 * r:32 * r + 32, :],
                                    wtile[0:32, :])
            xtile = xp.tile([128, 11664], f32r)
            for tf in range(6):
                nc.gpsimd.dma_start(xtile[:, tf * 1944:(tf + 1) * 1944],
                                    xq.ap()[:, tf * 1944:(tf + 1) * 1944])

            xv = xtile.rearrange("p (t d h w) -> p t d h w",
                                 t=6, d=6, h=18, w=18)

            # HAM warmup: ~60 matmuls on weight data only (independent of x
            # loads) so the PE clock is at 8/8 before the real work starts.
            wu = pp.tile([64, 512], f32, tag="ps", name="wu")
            for j in range(16):
                nc.tensor.matmul(wu[:, :], wtile[0:32, 0:64],
                                 wtile[0:32, 0:512],
                                 start=(j == 0), stop=(j == 15),
                                 tile_position=(0, 0))

            for to in range(4):
                for dp in range(2):
                    ps = [pp.tile([64, 512], f32, tag="ps",
                                  name=f"ps_{to}_{dp}_{r}") for r in range(4)]
                    for i, (kt, kd, kh, kw) in enumerate(TAPS):
                        for r in range(4):
                            rhs = xv[32 * r:32 * r + 32, to + kt,
                                     2 * dp + kd: 2 * dp + kd + 2,
                                     kh:kh + 16, kw:kw + 16]
                            lhsT = wtile[32 * r:32 * r + 32,
                                         i * 64:(i + 1) * 64]
                            nc.tensor.matmul(ps[r][:, :], lhsT, rhs,
                                             start=(i == 0), stop=(i == 80),
                                             tile_position=(32 * r, 0))
                    for r in range(4):
                        o = op_.tile([64, 512], f32, tag="ob",
                                     name=f"o_{to}_{dp}_{r}")
                        if r < 2:
                            nc.vector.tensor_scalar_add(o[:], ps[r][:, :],
                                                        btile[:, 0:1])
                        else:
                            nc.scalar.activation(
                                o[:], ps[r][:, :],
                                mybir.ActivationFunctionType.Identity,
                                bias=btile[:, 0:1])
                        off = to * 4096 + (4 * r + 2 * dp) * 256
                        nc.gpsimd.dma_start(out.ap()[:, off:off + 512], o[:])
    nc.compile()
    _NC = nc
    return nc


def _round_tf32(a):
    b = np.ascontiguousarray(a).view(np.uint32)
    r = (b + np.uint32(0x00000FFF) + ((b >> np.uint32(13)) & np.uint32(1))) \
        & np.uint32(0xFFFFE000)
    return r.view(np.float32)


def _prep_inputs(x, weight, bias):
    x = np.asarray(x, dtype=np.float32)
    weight = np.asarray(weight, dtype=np.float32)
    bias = np.asarray(bias, dtype=np.float32)

    w9 = weight.reshape(64, 32, 81).transpose(2, 1, 0)  # [tap, ci, co]
    warr = np.ascontiguousarray(w9.transpose(1, 0, 2)).reshape(32, 81 * 64)
    wq = _round_tf32(warr)
    bq = bias.reshape(64, 1).astype(np.float32)

    in_maps = []
    for b in range(2):
        xpad = np.pad(x[b], ((0, 0), (1, 1), (1, 1), (1, 1), (1, 1)))
        for tq in range(4):
            xt = xpad[:, 4 * tq:4 * tq + 6]  # [32, 6, 18, 18, 18]
            xqc = np.empty((128, 11664), np.float32)
            for r in range(4):
                xqc[32 * r:32 * r + 32] = \
                    xt[:, :, 4 * r:4 * r + 6].reshape(32, -1)
            in_maps.append({"xq": _round_tf32(xqc), "wq": wq, "biasq": bq})
    return in_maps


def run_spmd(x, weight, bias, trace=False, trace_cores=None, tmpdir=None):
    """Returns (output ndarray, BassKernelResults)."""
    from concourse.bass_utils import run_bass_kernel_spmd
    nc = _build()
    in_maps = _prep_inputs(x, weight, bias)
    res = run_bass_kernel_spmd(nc, in_maps, core_ids=list(range(N_CORES)),
                               trace=trace, trace_cores=trace_cores,
                               tmpdir=tmpdir)
    out = np.empty((2, 64, 16, 16, 16, 16), np.float32)
    for c in range(N_CORES):
        b, tq = c // 4, c % 4
        out[b, :, 4 * tq:4 * tq + 4] = \
            res.results[c]["out"].reshape(64, 4, 16, 16, 16)
    return out, res


def kernel(x, weight, bias):
    out, _ = run_spmd(x, weight, bias)
    return out


# revision 5
# speedup vs baseline: 1.0050x; 1.0050x over previous
"""Conv4d (B=2, Ci=32, Co=64, 16^4 spatial, k=3^4, stride 1, pad 1) on 8
Trainium2 NeuronCores.

Sharding: 8 cores = batch(2) x T-quarters(4). Each core computes
out[64co, 4t, 16d, 16h, 16w] for its (b, t-quarter).

Per-core layout: SBUF x tile [128, 6t*6d*324] where partition group
r in {0..3} holds ci=32 channels of the padded input restricted to the
D-halo window [4r, 4r+6) (plus T halo), planes flattened as 18x18=324.
The 4 partition groups process the 4 output-D-quarters concurrently via
PE row-group tiling (tile_position=(32r, 0)).

Each output (t, d-pair) plane-pair accumulates 81 tap matmuls
(K=32ci, M=64co, N=512=2d*16h*16w) in fp32r (TF32) into one PSUM bank
per row group; epilogue adds bias (DVE/ACT tensor_scalar) and DMAs out.
"""
import sys

sys.path.insert(0, "/opt/trn_rl_repo")
import numpy as np

N_CORES = 8
TAPS = [(kt, kd, kh, kw) for kt in range(3) for kd in range(3)
        for kh in range(3) for kw in range(3)]

_NC = None


def _build():
    global _NC
    if _NC is not None:
        return _NC
    import concourse.bacc as bacc
    import concourse.tile as tile
    from concourse import mybir

    f32 = mybir.dt.float32
    f32r = mybir.dt.float32r

    nc = bacc.Bacc("TRN2", debug=False, target_bir_lowering=False,
                   num_devices=N_CORES)
    xq = nc.dram_tensor("xq", [128, 6 * 6 * 324], f32r, kind="ExternalInput")
    wq = nc.dram_tensor("wq", [32, 81 * 64], f32r, kind="ExternalInput")
    bq = nc.dram_tensor("biasq", [64, 1], f32, kind="ExternalInput")
    out = nc.dram_tensor("out", [64, 16384], f32, kind="ExternalOutput")

    with tile.TileContext(nc) as tc:
        with tc.tile_pool(name="xp", bufs=1) as xp, \
             tc.tile_pool(name="wp", bufs=1) as wp, \
             tc.tile_pool(name="op", bufs=6) as op_, \
             tc.tile_pool(name="pp", bufs=8, space="PSUM") as pp:
            # weights first: they gate every matmul.  Load one 0.66MB copy,
            # replicate to the other 3 partition groups on-chip.
            wtile = wp.tile([128, 5184], f32r)
            nc.gpsimd.dma_start(wtile[0:32, :], wq.ap()[:])
            btile = wp.tile([64, 1], f32)
            nc.gpsimd.dma_start(btile[:], bq.ap()[:])
            for r in range(1, 4):
                nc.gpsimd.dma_start(wtile[32 * r:32 * r + 32, :],
                                    wtile[0:32, :])
            xtile = xp.tile([128, 11664], f32r)

            def load_frame(tf):
                nc.gpsimd.dma_start(xtile[:, tf * 1944:(tf + 1) * 1944],
                                    xq.ap()[:, tf * 1944:(tf + 1) * 1944])

            # round (to, *) reads t-frames to..to+2.  Frames 0-2 load up
            # front; frames 3-5 trickle in one round ahead of first use so
            # the bulk DMA never fights the PE for SBUF ports.
            for tf in range(3):
                load_frame(tf)

            xv = xtile.rearrange("p (t d h w) -> p t d h w",
                                 t=6, d=6, h=18, w=18)

            # HAM warmup: ~60 matmuls on weight data only (independent of x
            # loads) so the PE clock is at 8/8 before the real work starts.
            wu = pp.tile([64, 512], f32, tag="ps", name="wu")
            for j in range(16):
                nc.tensor.matmul(wu[:, :], wtile[0:32, 0:64],
                                 wtile[0:32, 0:512],
                                 start=(j == 0), stop=(j == 15),
                                 tile_position=(0, 0))

            for to in range(4):
                if to + 2 < 5:
                    load_frame(to + 3)
                for dp in range(2):
                    ps = [pp.tile([64, 512], f32, tag="ps",
                                  name=f"ps_{to}_{dp}_{r}") for r in range(4)]
                    for i, (kt, kd, kh, kw) in enumerate(TAPS):
                        for r in range(4):
                            rhs = xv[32 * r:32 * r + 32, to + kt,
                                     2 * dp + kd: 2 * dp + kd + 2,
                                     kh:kh + 16, kw:kw + 16]
                            lhsT = wtile[32 * r:32 * r + 32,
                                         i * 64:(i + 1) * 64]
                            nc.tensor.matmul(ps[r][:, :], lhsT, rhs,
                                             start=(i == 0), stop=(i == 80),
                                             tile_position=(32 * r, 0))
                    for r in range(4):
                        o = op_.tile([64, 512], f32, tag="ob",
                                     name=f"o_{to}_{dp}_{r}")
                        if r < 2:
                            nc.vector.tensor_scalar_add(o[:], ps[r][:, :],
                                                        btile[:, 0:1])
                        else:
                            nc.scalar.activation(
                                o[:], ps[r][:, :],
                                mybir.ActivationFunctionType.Identity,
                                bias=btile[:, 0:1])
                        off = to * 4096 + (4 * r + 2 * dp) * 256
                        nc.gpsimd.dma_start(out.ap()[:, off:off + 512], o[:])
    nc.compile()
    _NC = nc
    return nc


def _round_tf32(a):
    b = np.ascontiguousarray(a).view(np.uint32)
    r = (b + np.uint32(0x00000FFF) + ((b >> np.uint32(13)) & np.uint32(1))) \
        & np.uint32(0xFFFFE000)
    return r.view(np.float32)


def _prep_inputs(x, weight, bias):
    x = np.asarray(x, dtype=np.float32)
    weight = np.asarray(weight, dtype=np.float32)
    bias = np.asarray(bias, dtype=np.float32)

    w9 = weight.reshape(64, 32, 81).transpose(2, 1, 0)  # [tap, ci, co]
    warr = np.ascontiguousarray(w9.transpose(1, 0, 2)).reshape(32, 81 * 64)
    wq = _round_tf32(warr)
    bq = bias.reshape(64, 1).astype(np.float32)

    in_maps = []
    for b in range(2):
        xpad = np.pad(x[b], ((0, 0), (1, 1), (1, 1), (1, 1), (1, 1)))
        for tq in range(4):
            xt = xpad[:, 4 * tq:4 * tq + 6]  # [32, 6, 18, 18, 18]
            xqc = np.empty((128, 11664), np.float32)
            for r in range(4):
                xqc[32 * r:32 * r + 32] = \
                    xt[:, :, 4 * r:4 * r + 6].reshape(32, -1)
            in_maps.append({"xq": _round_tf32(xqc), "wq": wq, "biasq": bq})
    return in_maps


def run_spmd(x, weight, bias, trace=False, trace_cores=None, tmpdir=None):
    """Returns (output ndarray, BassKernelResults)."""
    from concourse.bass_utils import run_bass_kernel_spmd
    nc = _build()
    in_maps = _prep_inputs(x, weight, bias)
    res = run_bass_kernel_spmd(nc, in_maps, core_ids=list(range(N_CORES)),
                               trace=trace, trace_cores=trace_cores,
                               tmpdir=tmpdir)
    out = np.empty((2, 64, 16, 16, 16, 16), np.float32)
    for c in range(N_CORES):
        b, tq = c // 4, c % 4
        out[b, :, 4 * tq:4 * tq + 4] = \
            res.results[c]["out"].reshape(64, 4, 16, 16, 16)
    return out, res


def kernel(x, weight, bias):
    out, _ = run_spmd(x, weight, bias)
    return out
